# revision 1
# baseline (speedup 1.0000x reference)
"""Trainium2 Bass kernel for nn_DNN_sym_10101763080772 (moe_routing).

Network (all-linear, batch-1):
    g1  = x @ W1.T + b1          [128, 3]
    g12 = x @ W12.T + b12        [128, 3]
    g   = where(atom_list == 1, g1, g12)
    d   = (g.T @ x).reshape(9)
    h0  = d  @ Wl0.T + bl0       [8192]
    h1  = h0 @ Wl1.T + bl1       [8192]
    h2  = h1 @ Wl2.T + bl2       [8192]
    out = h2 @ Wo.T  + bo        [3]

Sharding over 8 cores (tensor parallel, no collectives):
  - embed/routing stage + h0 replicated on every core (tiny).
  - Wl1 row-sharded: core i computes h1[1024*i : 1024*(i+1)] exactly.
  - Wl2 column-sharded with the same slice: core i computes a partial h2.
  - Because the network is linear past that point, each core applies Wo to
    its partial h2 and returns a partial [3]; the host sums the 8 partials.
  - bl2 / bo are folded in on core 0 only (other cores get zero tensors).

All big matmuls use the "weights stationary, vector moving (N=1)"
orientation so every activation stays partition-major [128, C]; no
transposes are needed. Weights are pre-tiled on the host into
[128, 65536] slabs whose free dim is (mtile, ktile, m)-major, so the
kernel streams them with large contiguous DMAs straight into SBUF lhsT
tiles (sync-engine HWDGE ring). All small constants travel in one packed
blob on the scalar-engine HWDGE ring so they never delay the weight
stream. h0 is computed on the Vector engine (exact f32) to keep the
Tensor engine free for the streamed layers.
"""

import os
import sys

import numpy as np

if "/opt/trn_rl_repo" not in sys.path:
    sys.path.insert(0, "/opt/trn_rl_repo")

N_CORES = 8
NA = 128           # atoms
D = 8192           # hidden width
SH = D // N_CORES  # 1024 rows/cols per core

# "f32" (exact), "bf16" (half the HBM traffic), "f32r" (full-rate fp32 matmul)
BIG_DT = os.environ.get("KERNEL_DTYPE", "bf16")

# packed f32 constant blob column offsets
_C_X = 0          # [*, 0:3]   x
_C_ONES = 3       # [*, 3:4]   ones
_C_BL0 = 4        # [*, 4:68]  bl0 partition-major
_C_WL0 = 68       # [*, 68:644]  Wl0 k-major [p, k*64+c]
_C_BL1 = 644      # [*, 644:652] bl1 shard partition-major
_C_BL2 = 652      # [*, 652:716] bl2 (core0) partition-major
_C_WOT = 716      # [*, 716:908] Wo tiled [p, c*3+m]
_C_BO = 908       # [0:3, 908:909] bo (core0)
_C_ONESROW = 909  # [0:1, 909:1037] ones row (partition 0)
_C_W = 1037

_session = {}


def _build(big_dt_name):
    import concourse.bass as bass
    import concourse.mybir as mybir
    import concourse.tile as tile
    from concourse import bacc

    f32 = mybir.dt.float32
    i32 = mybir.dt.int32
    big_dt = {
        "f32": mybir.dt.float32,
        "f32r": mybir.dt.float32r,
        "bf16": mybir.dt.bfloat16,
    }[big_dt_name]
    # ~4 MB streamed chunks, 5 in flight: measured best (4MB/5 beats 8MB/3 —
    # prefetch slack matters more than fewer chunk-boundary handshakes)
    chunk_f = 16384 if big_dt_name == "bf16" else 8192
    n_bufs = 5 if big_dt_name == "bf16" else 4
    n_chunks = 65536 // chunk_f
    tiles_per_chunk = chunk_f // 128

    nc = bacc.Bacc("TRN2", target_bir_lowering=False, debug=False)

    blob128_d = nc.dram_tensor("blob128", [128, _C_W], f32, kind="ExternalInput")
    blob4_d = nc.dram_tensor("blob4", [4, 134], f32, kind="ExternalInput")
    atom_d = nc.dram_tensor("atom", [NA, 1], i32, kind="ExternalInput")
    l1w_d = nc.dram_tensor("l1w", [128, 65536], big_dt, kind="ExternalInput")
    l2w_d = nc.dram_tensor("l2w", [128, 65536], big_dt, kind="ExternalInput")
    q_d = nc.dram_tensor("q", [3, 1], f32, kind="ExternalOutput")

    add = mybir.AluOpType.add
    sub = mybir.AluOpType.subtract
    mult = mybir.AluOpType.mult
    is_eq = mybir.AluOpType.is_equal

    with tile.TileContext(nc) as tc:
        with (
            tc.tile_pool(name="const", bufs=1) as cp,
            tc.tile_pool(name="work", bufs=1) as wk,
            tc.tile_pool(name="wstream", bufs=n_bufs) as ws,
            tc.tile_pool(name="ps", bufs=1, space=bass.MemorySpace.PSUM) as pp,
        ):
            # ---- constants: 3 DMAs on the scalar HWDGE ring ----
            b128 = cp.tile([128, _C_W], f32)
            b4 = cp.tile([4, 134], f32)
            atom = cp.tile([NA, 1], i32)
            nc.scalar.dma_start(out=b128[:], in_=blob128_d[:])
            nc.scalar.dma_start(out=b4[:], in_=blob4_d[:])
            nc.scalar.dma_start(out=atom[:], in_=atom_d[:])

            x_sb = b128[:, _C_X : _C_X + 3]
            ones = b128[:, _C_ONES : _C_ONES + 1]
            bl0p = b128[:, _C_BL0 : _C_BL0 + 64]
            bl1p = b128[:, _C_BL1 : _C_BL1 + 8]
            bl2p = b128[:, _C_BL2 : _C_BL2 + 64]
            wot = b128[:, _C_WOT : _C_WOT + 192]
            bo = b128[0:3, _C_BO : _C_BO + 1]
            xTa = b4[:, 0:128]
            w1aug = b4[:, 128:131]
            w12aug = b4[:, 131:134]
            ones_row = b128[0:1, _C_ONESROW : _C_ONESROW + 128]

            # ---- routed embedding: g = select(atom==1, g1, g12) ----
            g1p = pp.tile([NA, 3], f32)
            g12p = pp.tile([NA, 3], f32)
            nc.tensor.matmul(g1p[:], xTa, w1aug, start=True, stop=True)
            nc.tensor.matmul(g12p[:], xTa, w12aug, start=True, stop=True)

            mask = wk.tile([NA, 1], f32)
            nc.vector.tensor_single_scalar(mask[:], atom[:], 1, is_eq)
            g12_sb = wk.tile([NA, 3], f32)
            nc.vector.tensor_copy(g12_sb[:], g12p[:])
            diff = wk.tile([NA, 3], f32)
            nc.vector.tensor_tensor(diff[:], g1p[:], g12_sb[:], sub)
            g_sb = wk.tile([NA, 3], f32)
            nc.vector.scalar_tensor_tensor(g_sb[:], diff[:], mask[:], g12_sb[:], mult, add)

            # ---- d = vec(g.T @ x): row form then broadcast to all partitions
            gx = wk.tile([NA, 9], f32)
            for a in range(3):
                nc.vector.tensor_scalar_mul(
                    gx[:, 3 * a : 3 * a + 3], x_sb, g_sb[:, a : a + 1]
                )
            drp = pp.tile([1, 9], f32)
            nc.tensor.matmul(drp[:], ones, gx[:], start=True, stop=True)
            drow = wk.tile([1, 9], f32)
            nc.vector.tensor_copy(drow[:], drp[:])
            dbp = pp.tile([128, 9], f32)
            nc.tensor.matmul(dbp[:], ones_row, drow[:], start=True, stop=True)
            dbc = wk.tile([128, 9], f32)
            nc.vector.tensor_copy(dbc[:], dbp[:])

            # ---- h0 = Wl0 @ d + bl0 on the Vector engine, [128, 64] ----
            acc_a = wk.tile([128, 64], f32)
            acc_b = wk.tile([128, 64], f32)
            h0 = wk.tile([128, 64], big_dt)
            cur, nxt = acc_a, acc_b
            nc.vector.scalar_tensor_tensor(
                cur[:], b128[:, _C_WL0 : _C_WL0 + 64], dbc[:, 0:1], bl0p, mult, add
            )
            for k in range(1, 9):
                dst = h0 if k == 8 else nxt
                nc.vector.scalar_tensor_tensor(
                    dst[:],
                    b128[:, _C_WL0 + 64 * k : _C_WL0 + 64 * (k + 1)],
                    dbc[:, k : k + 1],
                    cur[:],
                    mult,
                    add,
                )
                cur, nxt = nxt, cur

            # ---- layer 1 (row shard): h1_i = Wl1[rows] @ h0 + bl1[rows] ----
            # slab free index = mtile*8192 + ktile*128 + m ; tile t = mtile*64+ktile
            h1p = pp.tile([128, 8], f32)
            for c in range(n_chunks):
                wt = ws.tile([128, chunk_f], big_dt, tag="wchunk")
                nc.sync.dma_start(out=wt[:], in_=l1w_d[:, c * chunk_f : (c + 1) * chunk_f])
                for j in range(tiles_per_chunk):
                    t = c * tiles_per_chunk + j
                    mt, kt = divmod(t, 64)
                    nc.tensor.matmul(
                        h1p[:, mt : mt + 1],
                        wt[:, j * 128 : (j + 1) * 128],
                        h0[:, kt : kt + 1],
                        start=(kt == 0),
                        stop=(kt == 63),
                    )
            h1 = wk.tile([128, 8], big_dt)
            nc.vector.tensor_tensor(h1[:], h1p[:], bl1p, add)

            # ---- layer 2 (col shard): p2 = Wl2[:, cols] @ h1_i (+ bl2 core0)
            # slab free index = mtile2*1024 + kchunk*128 + m ; tile t = mtile2*8+kchunk
            # The final q = Wo @ p2 contraction is interleaved per chunk so no
            # work is left after the last weight byte lands; p2 PSUM ping-pongs
            # between two banks so the evacuating vector reads never collide
            # with the next chunk's matmul writes. The last chunks taper off in
            # size for the same reason.
            full = tiles_per_chunk
            taper = [full // 2, full // 4, full // 8, full // 8]
            taper = [t for t in taper if t >= 8] or [full]
            taper += [full - sum(taper)] if sum(taper) < full else []
            l2_chunks = [full] * (n_chunks - 1) + taper
            p2pa = pp.tile([128, full // 8], f32)
            p2pb = pp.tile([128, full // 8], f32)
            p2sb = wk.tile([128, 64], f32)
            qp = pp.tile([3, 1], f32)
            t0 = 0
            for ci, ntiles in enumerate(l2_chunks):
                wt = ws.tile([128, ntiles * 128], big_dt, tag="wchunk")
                nc.sync.dma_start(
                    out=wt[:], in_=l2w_d[:, t0 * 128 : (t0 + ntiles) * 128]
                )
                p2p = p2pa if ci % 2 == 0 else p2pb
                mt0 = t0 // 8
                nmt = ntiles // 8
                for j in range(ntiles):
                    t = t0 + j
                    mt, kc = divmod(t, 8)
                    nc.tensor.matmul(
                        p2p[:, mt - mt0 : mt - mt0 + 1],
                        wt[:, j * 128 : (j + 1) * 128],
                        h1[:, kc : kc + 1],
                        start=(kc == 0),
                        stop=(kc == 7),
                    )
                nc.vector.tensor_tensor(
                    p2sb[:, mt0 : mt0 + nmt],
                    p2p[:, 0:nmt],
                    bl2p[:, mt0 : mt0 + nmt],
                    add,
                )
                for ch in range(mt0, mt0 + nmt):
                    nc.tensor.matmul(
                        qp[:],
                        wot[:, ch * 3 : (ch + 1) * 3],
                        p2sb[:, ch : ch + 1],
                        start=(ch == 0),
                        stop=(ch == 63),
                    )
                t0 += ntiles

            q_sb = wk.tile([3, 1], f32)
            nc.vector.tensor_tensor(q_sb[:], qp[:], bo, add)
            nc.sync.dma_start(out=q_d[:], in_=q_sb[:])

    nc.compile()
    return nc


def _prep_in_maps(inputs, big_dt_name):
    import ml_dtypes

    big_np = np.dtype(ml_dtypes.bfloat16) if big_dt_name == "bf16" else np.float32

    f = lambda k: np.asarray(inputs[k], np.float32)
    x = f("x")
    W1, b1, W12, b12 = f("W1"), f("b1"), f("W12"), f("b12")
    Wl0, bl0 = f("Wl0"), f("bl0")
    Wl1, bl1 = f("Wl1"), f("bl1")
    Wl2, bl2 = f("Wl2"), f("bl2")
    Wo, bo = f("Wo"), f("bo")
    atom = np.asarray(inputs["atom_list"], np.int32).reshape(NA, 1)

    blob = np.zeros((128, _C_W), np.float32)
    blob[:, _C_X : _C_X + 3] = x
    blob[:, _C_ONES] = 1.0
    blob[:, _C_BL0 : _C_BL0 + 64] = bl0.reshape(64, 128).T
    # Wl0 k-major: [p, k*64 + c] = Wl0[c*128+p, k]
    blob[:, _C_WL0 : _C_WL0 + 576] = (
        Wl0.reshape(64, 128, 9).transpose(1, 2, 0).reshape(128, 576)
    )
    blob[:, _C_BL2 : _C_BL2 + 64] = bl2.reshape(64, 128).T  # zeroed for cores 1-7
    blob[:, _C_WOT : _C_WOT + 192] = (
        Wo.reshape(3, 64, 128).transpose(2, 1, 0).reshape(128, 192)
    )
    blob[0:3, _C_BO] = bo
    blob[0, _C_ONESROW : _C_ONESROW + 128] = 1.0

    blob4 = np.zeros((4, 134), np.float32)
    blob4[0:3, 0:128] = x.T
    blob4[3, 0:128] = 1.0
    blob4[0:3, 128:131] = W1.T
    blob4[3, 128:131] = b1
    blob4[0:3, 131:134] = W12.T
    blob4[3, 131:134] = b12

    Wl1b = Wl1.astype(big_np)  # cast before relayout: halves the shuffle bytes
    Wl2b = Wl2.astype(big_np)
    in_maps = []
    for i in range(N_CORES):
        rows = slice(SH * i, SH * (i + 1))
        l1w = np.ascontiguousarray(
            Wl1b[rows].reshape(8, 128, 64, 128).transpose(3, 0, 2, 1).reshape(128, 65536)
        )
        l2w = np.ascontiguousarray(
            Wl2b[:, rows].reshape(64, 128, 8, 128).transpose(3, 0, 2, 1).reshape(128, 65536)
        )
        b = blob.copy()
        b[:, _C_BL1 : _C_BL1 + 8] = bl1[rows].reshape(8, 128).T
        if i != 0:
            b[:, _C_BL2 : _C_BL2 + 64] = 0.0
            b[0:3, _C_BO] = 0.0
        in_maps.append({"blob128": b, "blob4": blob4, "atom": atom, "l1w": l1w, "l2w": l2w})
    return in_maps


def _install_profile_shim():
    """Make trace=True work under axon: provide the antenv.axon_hooks
    registry this container's antenv stub lacks, wired to the ctypes NTFF
    profiler from trn_agent_boot."""
    import types

    try:
        from antenv.axon_hooks import get_axon_ntff_profile_hook  # noqa: F401
        return
    except ImportError:
        pass
    try:
        import antenv
        from trn_agent_boot.trn_boot import _ntff_profile_via_ctypes

        mod = types.ModuleType("antenv.axon_hooks")
        holder = {"h": None}
        mod.set_axon_ntff_profile_hook = lambda h: holder.__setitem__("h", h)
        mod.get_axon_ntff_profile_hook = lambda: holder["h"]
        sys.modules["antenv.axon_hooks"] = mod
        antenv.axon_hooks = mod
        mod.set_axon_ntff_profile_hook(
            _ntff_profile_via_ctypes("/opt/axon/libaxon_pjrt.so")
        )
    except Exception as e:  # profiling is best-effort only
        print(f"profile shim unavailable: {e}")


def kernel(**inputs) -> np.ndarray:
    from concourse import bass_utils

    big = BIG_DT
    if big not in _session:
        _session[big] = _build(big)
    nc = _session[big]

    in_maps = _prep_in_maps(inputs, big)
    trace = os.environ.get("KERNEL_TRACE", "0") == "1"
    if trace:
        _install_profile_shim()
    res = bass_utils.run_bass_kernel_spmd(
        nc, in_maps, core_ids=list(range(N_CORES)), trace=trace
    )
    if trace and res.exec_time_ns is not None:
        print(f"HW exec time: {res.exec_time_ns} ns")
        kernel.last_exec_time_ns = res.exec_time_ns
    kernel.last_results = res

    out = np.zeros(3, np.float64)
    for r in res.results:
        out += r["q"][:, 0].astype(np.float64)
    return out.astype(np.float32)

